# revision 2
# baseline (speedup 1.0000x reference)
"""Multi-head attention (B=8, P=1024, D=768, H=12) on 8 TRN2 NeuronCores.

Strategy: pure data parallelism — batch element b runs on core b (no
collectives). Host pre-transposes x and casts operands to bf16; each core
computes QK^T/softmax/AV/proj for its batch element with all matmuls on the
TensorEngine (bf16, fp32 PSUM accumulation), exp on the ScalarEngine, and
evacuations/normalization on the VectorEngine.

Schedule (v2): the attention loop is organized as 6 per-pair "windows" paced
by the ScalarEngine exp stream. Per k-tile, the two heads' S^T matmuls write
two [128,1024] PSUM tiles (even head at PE rows 0-63, odd at 64-127, so
adjacent matmuls occupy distinct row-groups and overlap in the array), each
evacuated by ONE [128,1024] exp (wider activations amortize the ~185ns
SBUF-ack overhead). exp outputs (pt) are held in SBUF for a full window; the
AV matmuls for pair p run as PE filler inside pair p+1's window, one
(head, j-half) accumulation group at a time into a single PSUM bank — this
frees enough PSUM for the double-size S tiles. Remaining PE filler: the next
pair's qkT feature tiles, the v-projection (window 0), and the per-head
normalization broadcasts. Softmax reciprocals run on the idle VectorEngine
(InstReciprocal) instead of the ScalarEngine ln/exp chain so they don't
steal exp throughput. The tail does per-q-tile wa transposes fused with the
output projection, as before.

Self-contained: builds + compiles the Bass kernel on first call, runs via
PJRT (axon) across cores 0-7, and reassembles full outputs. Returns the
tuple (out, weighted_avg), matching the reference.
"""

import numpy as np
from collections import deque
from contextlib import ExitStack

import bass_rust
import concourse.bass as bass
import concourse.tile as tile
from concourse import mybir
from concourse import bass2jax

B, P, D = 8, 1024, 768
H = 12
HD = D // H            # 64
SCALE = HD ** -0.5     # 0.125
N_CORES = 8
KT = D // 128          # 6 contraction tiles over d
QT = P // 128          # 8 tiles over sequence
BF = mybir.dt.bfloat16
F32 = mybir.dt.float32
NP_BF16 = np.dtype(mybir.dt.np(BF))

IN_NAMES = ["xT", "wqk", "wv", "wph", "bqk", "bv", "bp", "ident", "selmat"]
OUT_NAMES = ["out", "wa"]


def _split_excess_waits(nc, max_waits=1):
    """This container's walrus build rejects instructions carrying more than
    one sync wait. Hoist excess waits onto same-engine no-ops inserted just
    before the overloaded instruction (engine queues execute in order, so
    wait-for-all-before-exec semantics are preserved)."""
    ctr = 0
    for bb in nc.main_func.blocks:
        newlist = []
        dirty = False
        for inst in bb.instructions:
            si = inst.sync_info
            waits = list(si.on_wait) if (si is not None and si.on_wait) else []
            if len(waits) > max_waits:
                excess, keep = waits[:-max_waits], waits[-max_waits:]
                for i in range(0, len(excess), max_waits):
                    chunk = excess[i : i + max_waits]
                    nop = bass_rust.InstNoOp(name=f"WSPILL-{ctr}")
                    ctr += 1
                    nop.engine = inst.engine
                    nop.sync_info = bass_rust.SyncInfo(on_wait=chunk, on_update=[])
                    newlist.append(nop)
                inst.sync_info = bass_rust.SyncInfo(
                    on_wait=keep, on_update=list(si.on_update or [])
                )
                dirty = True
            newlist.append(inst)
        if dirty:
            bb.instructions = newlist
    return ctr


def _prune_implied_waits(nc):
    """Drop transitively-implied waits: when an instruction carries two waits
    whose satisfying semaphore updates are both posted by the SAME in-order
    engine queue (non-DMA, sem-inc only), the wait satisfied earlier in that
    queue is implied by the later one — in-order engines post updates in
    order. This removes most multi-wait spills so the sequencers don't
    execute a wait no-op before every instruction."""
    from collections import defaultdict

    sem_updates = defaultdict(list)  # sem id -> [(engine, pos, cum_value)]
    sem_engines = defaultdict(set)
    sem_unsafe = defaultdict(bool)
    engine_pos = defaultdict(int)
    cum = defaultdict(int)
    for bb in nc.main_func.blocks:
        for inst in bb.instructions:
            eng = inst.engine
            pos = engine_pos[eng]
            engine_pos[eng] = pos + 1
            si = inst.sync_info
            if si is None or not si.on_update:
                continue
            is_dma = isinstance(inst, bass_rust.InstDMACopy)
            for u in si.on_update:
                if u.sync_type != "semaphore" or u.update_mode != "sem-inc":
                    sem_unsafe[u.id] = True
                    continue
                cum[u.id] += u.update_value
                sem_updates[u.id].append((eng, pos, cum[u.id]))
                sem_engines[u.id].add(eng)
                if is_dma:
                    sem_unsafe[u.id] = True

    def satisfier(w):
        """(engine, queue_pos) of the update that first satisfies wait w, or
        None when the sem is DMA-completed, multi-queue, or unanalyzable."""
        if w.sync_type != "semaphore" or w.wait_mode != "sem-ge-imm":
            return None
        if sem_unsafe[w.id] or len(sem_engines[w.id]) != 1:
            return None
        for eng, pos, val in sem_updates[w.id]:
            if val >= w.wait_value:
                return (eng, pos)
        return None

    pruned = 0
    engine_pos = defaultdict(int)
    for bb in nc.main_func.blocks:
        for inst in bb.instructions:
            own = (inst.engine, engine_pos[inst.engine])
            engine_pos[inst.engine] = own[1] + 1
            si = inst.sync_info
            waits = list(si.on_wait) if (si is not None and si.on_wait) else []
            if not waits:
                continue
            sats = [satisfier(w) for w in waits]
            keep = []
            for i, (w, s) in enumerate(zip(waits, sats)):
                # implied by the instruction's own in-order engine: the
                # satisfying update was posted by an earlier instruction on
                # this queue, which has completed by the time we dispatch
                implied = s is not None and s[0] == own[0] and s[1] < own[1]
                # implied by another wait satisfied later on the same queue
                implied = implied or (
                    s is not None
                    and any(
                        j != i
                        and sj is not None
                        and sj[0] == s[0]
                        and (sj[1] > s[1] or (sj[1] == s[1] and j < i))
                        for j, sj in enumerate(sats)
                    )
                )
                if implied:
                    pruned += 1
                else:
                    keep.append(w)
            if len(keep) < len(waits):
                inst.sync_info = bass_rust.SyncInfo(
                    on_wait=keep, on_update=list(si.on_update or [])
                )
    return pruned


def _bcast_ap(dram_ap, parts):
    """Partition-stride-0 DMA source view of a 1-D DRAM tensor: [n] -> [parts, n]."""
    return bass.AP(
        tensor=dram_ap.tensor,
        offset=dram_ap.offset,
        ap=[[0, parts]] + list(dram_ap.ap),
    )


def build_nc(split_waits=True, loop_n=None, unroll=1):
    nc = bass.Bass(target_bir_lowering=False)

    xT_e = nc.declare_dram_parameter("xT", [D, P], BF, isOutput=False)
    wqk_e = nc.declare_dram_parameter("wqk", [D, 2 * D], BF, isOutput=False)
    wv_e = nc.declare_dram_parameter("wv", [D, D], BF, isOutput=False)
    wph_e = nc.declare_dram_parameter("wph", [H // 2, 128, D], BF, isOutput=False)
    bqk_e = nc.declare_dram_parameter("bqk", [128, 2 * D // 128], F32, isOutput=False)
    bv_e = nc.declare_dram_parameter("bv", [D], F32, isOutput=False)
    bp_e = nc.declare_dram_parameter("bp", [D], F32, isOutput=False)
    id_e = nc.declare_dram_parameter("ident", [128, 128], BF, isOutput=False)
    sel_e = nc.declare_dram_parameter("selmat", [H, H * HD], BF, isOutput=False)
    out_e = nc.declare_dram_parameter("out", [P, D], BF, isOutput=True)
    wa_e = nc.declare_dram_parameter("wa", [P, D], BF, isOutput=True)

    EXP = mybir.ActivationFunctionType.Exp

    with tile.TileContext(nc) as tc, ExitStack() as ctx:
        if loop_n is not None:
            ctx.enter_context(tc.For_i(0, loop_n, 1))
        const = ctx.enter_context(tc.tile_pool(name="const", bufs=1))
        qkp = ctx.enter_context(tc.tile_pool(name="qkp", bufs=1))
        vxp = ctx.enter_context(tc.tile_pool(name="vxp", bufs=1))
        wtp = ctx.enter_context(tc.tile_pool(name="wtp", bufs=1))
        ptp = ctx.enter_context(tc.tile_pool(name="ptp", bufs=28))
        stgp = ctx.enter_context(tc.tile_pool(name="stgp", bufs=1))
        outp = ctx.enter_context(tc.tile_pool(name="outp", bufs=4))
        # PSUM budget (8 banks): psS = 2 x [128,1024] S tiles (4 banks),
        # psAV = 2 x [128,512] (AV accumulation groups + upfront qkT units,
        # 2 banks), psX = 2 x [128,512] filler/psr/proj utility (2 banks).
        psS = ctx.enter_context(tc.tile_pool(name="psS", bufs=2, space="PSUM"))
        psAV = ctx.enter_context(tc.tile_pool(name="psAV", bufs=2, space="PSUM"))
        psX = ctx.enter_context(tc.tile_pool(name="psX", bufs=2, space="PSUM"))

        for _it in range(unroll):
            # ---- constant loads --------------------------------------------
            xT = [const.tile([128, P], BF, tag=f"xT{k}", name=f"xT{k}") for k in range(KT)]
            wqk = [const.tile([128, 2 * D], BF, tag=f"wqk{k}", name=f"wqk{k}") for k in range(KT)]
            wv = [const.tile([128, D], BF, tag=f"wv{k}", name=f"wv{k}") for k in range(KT)]
            wp = [const.tile([128, D], BF, tag=f"wp{p}", name=f"wp{p}") for p in range(H // 2)]
            bqk = const.tile([128, 2 * D // 128], F32, tag="bqk", name="bqk")
            bvb = const.tile([128, D], F32, tag="bvb", name="bvb")
            bpb = const.tile([128, D], F32, tag="bpb", name="bpb")
            ident = const.tile([128, 128], BF, tag="ident", name="ident")
            selmat = const.tile([H, H * HD], BF, tag="selmat", name="selmat")

            # DGE rings are in-order with head-of-line blocking, so queues are
            # segregated by readiness: needed-early input loads on SP,
            # mid-kernel small transfers on Pool, output writebacks on Act.
            for k in range(KT):
                nc.sync.dma_start(out=xT[k], in_=xT_e[k * 128 : (k + 1) * 128, :])
                nc.sync.dma_start(out=wqk[k], in_=wqk_e[k * 128 : (k + 1) * 128, :])
                nc.gpsimd.dma_start(out=wv[k], in_=wv_e[k * 128 : (k + 1) * 128, :])
            nc.gpsimd.dma_start(out=bqk, in_=bqk_e[:])
            nc.gpsimd.dma_start(out=bvb, in_=_bcast_ap(bv_e[:], 128))
            nc.gpsimd.dma_start(out=selmat, in_=sel_e[:])
            for p in range(H // 2):
                nc.gpsimd.dma_start(out=wp[p], in_=wph_e[p])
            nc.gpsimd.dma_start(out=bpb, in_=_bcast_ap(bp_e[:], 128))
            nc.gpsimd.dma_start(out=ident, in_=id_e[:])

            # ---- phase 1: qT / kT = (w_qk)^T @ x^T  [feature-major] --------
            # qkT[m] rows = features m*128..; m 0..5 -> q, 6..11 -> k.
            # Pair pr's S needs tiles {pr, 6+pr}; the remaining tiles are
            # emitted via the filler stream inside the attention windows.
            qkT = [qkp.tile([128, P], BF, tag=f"qkT{m}", name=f"qkT{m}") for m in range(2 * D // 128)]

            def qkT_closures(ms, pool=None):
                """One closure per matmul; the last of each (m, j) unit also
                emits the DVE bias-add evacuation into the qkT tile."""
                cls = []
                for m in ms:
                    for j in range(2):
                        st = {}

                        def mk(k, m=m, j=j, st=st):
                            def go():
                                if k == 0:
                                    st["ps"] = (pool or psX).tile(
                                        [128, 512], F32,
                                        tag="px" if pool is None else "av", name="px")
                                nc.tensor.matmul(
                                    st["ps"],
                                    lhsT=wqk[k][:, m * 128 : (m + 1) * 128],
                                    rhs=xT[k][:, j * 512 : (j + 1) * 512],
                                    start=(k == 0),
                                    stop=(k == KT - 1),
                                )
                                if k == KT - 1:
                                    nc.vector.tensor_scalar_add(
                                        qkT[m][:, j * 512 : (j + 1) * 512],
                                        st["ps"],
                                        bqk[:, m : m + 1],
                                    )
                            return go

                        cls.extend(mk(k) for k in range(KT))
                return cls

            # upfront units (pair 0's q/k tiles) draw from psAV, whose last
            # prior use (the previous iteration's AV(5) groups) clears early
            # in that iteration's tail
            for cl in qkT_closures([0, 6], pool=psAV):
                cl()

            # ---- phase 2: v natural [seq-major] with ones column -----------
            # vext[p][:, h, 0:64] = v_h rows p*128..; vext[p][:, h, 64] = 1.0
            # Emitted as filler closures inside window 0 (memsets upfront).
            vext = [vxp.tile([128, H, HD + 1], BF, tag=f"vext{p}", name=f"vext{p}") for p in range(QT)]
            for p in range(QT):
                nc.vector.memset(vext[p][:, :, HD : HD + 1], 1.0)

            def vext_closures():
                cls = []
                for p in range(QT):
                    for (c0, cw) in ((0, 512), (512, 256)):
                        st = {}

                        def mk(k, p=p, c0=c0, cw=cw, st=st):
                            def go():
                                if k == 0:
                                    st["ps"] = psX.tile([128, 512], F32, tag="px", name="px")
                                nc.tensor.matmul(
                                    st["ps"][:, :cw],
                                    lhsT=xT[k][:, p * 128 : (p + 1) * 128],
                                    rhs=wv[k][:, c0 : c0 + cw],
                                    start=(k == 0),
                                    stop=(k == KT - 1),
                                )
                                if k == KT - 1:
                                    nh = cw // HD
                                    nc.vector.tensor_add(
                                        vext[p][:, c0 // HD : c0 // HD + nh, 0:HD],
                                        st["ps"][:, :cw].rearrange("p (h d) -> p h d", d=HD),
                                        bvb[:, c0 : c0 + cw].rearrange("p (h d) -> p h d", d=HD),
                                    )
                            return go

                        cls.extend(mk(k) for k in range(KT))
                return cls

            # ---- phase 3: six per-pair windows -----------------------------
            # Window p: per k-tile, S^T for heads (2p, 2p+1) into two
            # [128,1024] PSUM tiles (even head stationary at PE rows 0-63,
            # odd at 64-127 -> adjacent matmuls in distinct row-groups), one
            # [128,1024] exp each. pt tiles persist in SBUF; AV for pair p
            # runs as filler in window p+1, one (head, j) group at a time.
            waTp = [wtp.tile([128, P], BF, tag=f"waTp{p}", name=f"waTp{p}") for p in range(H // 2)]
            dens12 = stgp.tile([H, P], BF, tag="dens12", name="dens12")
            recip12 = stgp.tile([H, P], F32, tag="recip12", name="recip12")
            recip12b = stgp.tile([H, P], BF, tag="recip12b", name="recip12b")
            nc.vector.memset(dens12, 1.0)
            pt_tiles = {}   # (h, kt) -> SBUF [128, 1024] bf16
            stg_tiles = {}  # (h, j) -> SBUF [65, 512] bf16

            def av_closures(pr):
                """AV for pair pr: per (head, j-half) one 8-matmul PSUM
                accumulation group, evacuated to stg; the denominator row
                rides partition 64 and is copied into dens12 by DMA."""
                cls = []
                for h in (2 * pr, 2 * pr + 1):
                    for j in range(2):
                        st = {}

                        def mk(kt, h=h, j=j, st=st):
                            def go():
                                if kt == 0:
                                    st["ps"] = psAV.tile([128, 512], F32, tag="av", name="av")
                                nc.tensor.matmul(
                                    st["ps"][: HD + 1, :],
                                    lhsT=vext[kt][:, h, :],
                                    rhs=pt_tiles[(h, kt)][:, j * 512 : (j + 1) * 512],
                                    start=(kt == 0),
                                    stop=(kt == QT - 1),
                                )
                                if kt == QT - 1:
                                    stg = stgp.tile(
                                        [HD + 1, 512], BF, tag=f"stg{h}j{j}", name=f"stg{h}j{j}"
                                    )
                                    nc.vector.tensor_copy(stg, st["ps"][: HD + 1, :])
                                    nc.sync.dma_start(
                                        out=dens12[h : h + 1, j * 512 : (j + 1) * 512],
                                        in_=stg[HD : HD + 1, :],
                                    )
                                    stg_tiles[(h, j)] = stg
                            return go

                        cls.extend(mk(kt) for kt in range(QT))
                return cls

            def recip_batch():
                """Softmax reciprocals on the idle VectorEngine (keeps the
                ScalarEngine free for the exp stream)."""
                nc.vector.reciprocal(recip12[0:H, :], dens12[0:H, :])
                nc.vector.tensor_copy(recip12b[0:H, :], recip12[0:H, :])

            def norm_closures(heads):
                """Per (h, j): recip broadcast via selector matmul, then DVE
                multiply of the staged AV rows; odd heads merge into the pair
                tile's upper partitions by DMA."""
                cls = []
                for h in heads:
                    for j in range(2):
                        def go(h=h, j=j):
                            psr = psX.tile([HD, 512], F32, tag="px", name="psr")
                            nc.tensor.matmul(
                                psr,
                                lhsT=selmat[:, h * HD : (h + 1) * HD],
                                rhs=recip12b[0:H, j * 512 : (j + 1) * 512],
                                start=True,
                                stop=True,
                            )
                            sl = slice(j * 512, (j + 1) * 512)
                            if h % 2 == 0:
                                nc.vector.tensor_mul(
                                    waTp[h // 2][0:HD, sl], stg_tiles[(h, j)][0:HD, :], psr
                                )
                            else:
                                wt = outp.tile([HD, 512], BF, tag="wtmp", name="wtmp")
                                nc.vector.tensor_mul(wt, stg_tiles[(h, j)][0:HD, :], psr)
                                nc.sync.dma_start(out=waTp[h // 2][HD:128, sl], in_=wt)
                        cls.append(go)
                return cls

            filler = deque()
            filler.extend(vext_closures())
            for pr in range(H // 2):
                heads = (2 * pr, 2 * pr + 1)
                if pr + 1 < H // 2:
                    filler.extend(qkT_closures([pr + 1, 6 + pr + 1]))
                if pr >= 1:
                    filler.extend(av_closures(pr - 1))
                if pr == 5:
                    # recip batch 1 (heads 0-7) is emitted mid-window-5 below;
                    # norm for pairs 0-2 can follow as late window-5 filler
                    filler.extend(norm_closures(range(0, 6)))
                for kt in range(QT):
                    pss = {}
                    for h in heads:
                        pss[h] = psS.tile([128, 1024], F32, tag="ss", name="ss")
                    for j in range(2):
                        for h in heads:
                            base = (h % 2) * 64
                            nc.tensor.matmul(
                                pss[h][:, j * 512 : (j + 1) * 512],
                                lhsT=qkT[6 + pr][base : base + 64, kt * 128 : (kt + 1) * 128],
                                rhs=qkT[pr][base : base + 64, j * 512 : (j + 1) * 512],
                                start=True,
                                stop=True,
                            )
                    for h in heads:
                        pt = ptp.tile([128, 1024], BF, tag="pt", name="pt")
                        nc.scalar.activation(pt, pss[h], EXP, scale=SCALE)
                        pt_tiles[(h, kt)] = pt
                    if pr == 5 and kt == 3:
                        # heads 0-7 denominators are complete (AV(3) ran in
                        # window 4); batch the reciprocals now so norm(0-2)
                        # can run as this window's late filler
                        recip_batch()
                    # filler pacing: ~4 single-matmul closures per k-tile
                    # covers the ACT-vs-PE slack (exp 2.08us vs S 0.43us)
                    for _ in range((5, 5, 5, 5, 5, 5, 5, 4)[kt]):
                        if filler:
                            filler.popleft()()

            # ---- tail: AV(5), late norms, wa transposes fused with proj ----
            for cl in av_closures(5):
                cl()
            while filler:
                filler.popleft()()
            recip_batch()
            for cl in norm_closures(range(6, H)):
                cl()

            def emit_transposes(qt, prs, psw):
                for p in prs:
                    nc.tensor.matmul(
                        psw[:, p * 128 : (p + 1) * 128],
                        lhsT=waTp[p][:, qt * 128 : (qt + 1) * 128],
                        rhs=ident,
                        start=True,
                        stop=True,
                    )

            def emit_proj(qt, ps, c0, cw, prs, start, stop):
                for i, p in enumerate(prs):
                    nc.tensor.matmul(
                        ps[:, c0 : c0 + cw],
                        lhsT=waTp[p][:, qt * 128 : (qt + 1) * 128],
                        rhs=wp[p][:, c0 : c0 + cw],
                        start=start and i == 0,
                        stop=stop and i == len(prs) - 1,
                        skip_group_check=True,
                    )

            def emit_evacs(qt, psw, ps):
                wa_sb = outp.tile([128, D], BF, tag="wa_sb", name="wa_sb")
                nc.scalar.copy(wa_sb, psw[:, :D])
                nc.scalar.dma_start(out=wa_e[qt * 128 : (qt + 1) * 128, :], in_=wa_sb)
                out_sb = outp.tile([128, D], BF, tag="out_sb", name="out_sb")
                nc.vector.tensor_add(out_sb, ps[:, :D], bpb)
                nc.scalar.dma_start(out=out_e[qt * 128 : (qt + 1) * 128, :], in_=out_sb)

            # per q-tile: one [128,1024] psS tile holds the wa transpose
            # (cols 0-767), a second holds the proj accumulation (cols 0-767)
            for qt in range(QT):
                psw = psS.tile([128, 1024], F32, tag="ss", name="psw")
                emit_transposes(qt, range(H // 2), psw)
                ps = psS.tile([128, 1024], F32, tag="ss", name="psp")
                for (c0, cw) in ((0, 512), (512, 256)):
                    emit_proj(qt, ps, c0, cw, range(H // 2), True, True)
                emit_evacs(qt, psw, ps)

    if split_waits:
        _prune_implied_waits(nc)
        _split_excess_waits(nc)
    return nc


def make_in_maps(x, w_qkv, b_qkv, w_proj, b_proj):
    """Host-side shard prep: batch element b -> core b; weights replicated."""
    xf = np.asarray(x, dtype=np.float32)
    wqkv = np.asarray(w_qkv, dtype=np.float32)
    bqkv = np.asarray(b_qkv, dtype=np.float32)
    wproj = np.asarray(w_proj, dtype=np.float32)
    bproj = np.asarray(b_proj, dtype=np.float32)

    wqk = np.ascontiguousarray(wqkv[:, : 2 * D]).astype(NP_BF16)
    wv = np.ascontiguousarray(wqkv[:, 2 * D :]).astype(NP_BF16)
    wph = np.ascontiguousarray(wproj.reshape(H // 2, 128, D)).astype(NP_BF16)
    bqk = np.ascontiguousarray(bqkv[: 2 * D].reshape(2 * D // 128, 128).T)
    bv = np.ascontiguousarray(bqkv[2 * D :])
    ident = np.eye(128, dtype=np.float32).astype(NP_BF16)
    selmat = np.kron(np.eye(H, dtype=np.float32), np.ones((1, HD), np.float32)).astype(NP_BF16)

    in_maps = []
    for b in range(N_CORES):
        in_maps.append(
            {
                "xT": np.ascontiguousarray(xf[b].T).astype(NP_BF16),
                "wqk": wqk,
                "wv": wv,
                "wph": wph,
                "bqk": bqk,
                "bv": bv,
                "bp": bproj,
                "ident": ident,
                "selmat": selmat,
            }
        )
    return in_maps


_CACHE = {}


def _get_nc():
    if "nc" not in _CACHE:
        _CACHE["nc"] = build_nc()
    return _CACHE["nc"]


def run_once(in_maps, nc=None):
    """One 8-core execution via the PJRT redirect path (fresh jit per call;
    NEFF comes from the neuron compile cache after the first call)."""
    if nc is None:
        nc = _get_nc()
    return bass2jax.run_bass_via_pjrt(nc, in_maps, n_cores=N_CORES)


def kernel(x, w_qkv, b_qkv, w_proj, b_proj):
    in_maps = make_in_maps(x, w_qkv, b_qkv, w_proj, b_proj)
    results = run_once(in_maps)
    out = np.stack([results[b]["out"] for b in range(N_CORES)]).astype(np.float32)
    wa = np.stack([results[b]["wa"] for b in range(N_CORES)]).astype(np.float32)
    return (out, wa)


# revision 7
# speedup vs baseline: 1.0889x; 1.0889x over previous
"""Multi-head attention (B=8, P=1024, D=768, H=12) on 8 TRN2 NeuronCores.

Strategy: pure data parallelism — batch element b runs on core b (no
collectives). Host pre-transposes x and casts operands to bf16; each core
computes QK^T/softmax/AV/proj for its batch element with all matmuls on the
TensorEngine (bf16, fp32 PSUM accumulation), exp on the ScalarEngine, and
evacuations/normalization on the VectorEngine.

Schedule (v2): the attention loop is organized as 6 per-pair "windows" paced
by the ScalarEngine exp stream. Per k-tile, the two heads' S^T matmuls write
two [128,1024] PSUM tiles (even head at PE rows 0-63, odd at 64-127, so
adjacent matmuls occupy distinct row-groups and overlap in the array), each
evacuated by ONE [128,1024] exp (wider activations amortize the ~185ns
SBUF-ack overhead). exp outputs (pt) are held in SBUF for a full window; the
AV matmuls for pair p run as PE filler inside pair p+1's window, one
(head, j-half) accumulation group at a time into a single PSUM bank — this
frees enough PSUM for the double-size S tiles. Remaining PE filler: the next
pair's qkT feature tiles, the v-projection (window 0), and the per-head
normalization broadcasts. Softmax reciprocals run on the idle VectorEngine
(InstReciprocal) instead of the ScalarEngine ln/exp chain so they don't
steal exp throughput. The tail does per-q-tile wa transposes fused with the
output projection, as before.

Self-contained: builds + compiles the Bass kernel on first call, runs via
PJRT (axon) across cores 0-7, and reassembles full outputs. Returns the
tuple (out, weighted_avg), matching the reference.
"""

import numpy as np
from collections import deque
from contextlib import ExitStack

import bass_rust
import concourse.bass as bass
import concourse.tile as tile
from concourse import mybir
from concourse import bass2jax

B, P, D = 8, 1024, 768
H = 12
HD = D // H            # 64
SCALE = HD ** -0.5     # 0.125
N_CORES = 8
KT = D // 128          # 6 contraction tiles over d
QT = P // 128          # 8 tiles over sequence
BF = mybir.dt.bfloat16
F32 = mybir.dt.float32
NP_BF16 = np.dtype(mybir.dt.np(BF))

IN_NAMES = ["xT", "wqk", "wv", "wph", "bqk", "bv", "bp", "ident", "selmat"]
OUT_NAMES = ["out", "wa"]


def _split_excess_waits(nc, max_waits=1):
    """This container's walrus build rejects instructions carrying more than
    one sync wait. Hoist excess waits onto same-engine no-ops inserted just
    before the overloaded instruction (engine queues execute in order, so
    wait-for-all-before-exec semantics are preserved)."""
    ctr = 0
    for bb in nc.main_func.blocks:
        newlist = []
        dirty = False
        for inst in bb.instructions:
            si = inst.sync_info
            waits = list(si.on_wait) if (si is not None and si.on_wait) else []
            if len(waits) > max_waits:
                excess, keep = waits[:-max_waits], waits[-max_waits:]
                for i in range(0, len(excess), max_waits):
                    chunk = excess[i : i + max_waits]
                    nop = bass_rust.InstNoOp(name=f"WSPILL-{ctr}")
                    ctr += 1
                    nop.engine = inst.engine
                    nop.sync_info = bass_rust.SyncInfo(on_wait=chunk, on_update=[])
                    newlist.append(nop)
                inst.sync_info = bass_rust.SyncInfo(
                    on_wait=keep, on_update=list(si.on_update or [])
                )
                dirty = True
            newlist.append(inst)
        if dirty:
            bb.instructions = newlist
    return ctr


def _prune_implied_waits(nc):
    """Drop transitively-implied waits: when an instruction carries two waits
    whose satisfying semaphore updates are both posted by the SAME in-order
    engine queue (non-DMA, sem-inc only), the wait satisfied earlier in that
    queue is implied by the later one — in-order engines post updates in
    order. This removes most multi-wait spills so the sequencers don't
    execute a wait no-op before every instruction."""
    from collections import defaultdict

    sem_updates = defaultdict(list)  # sem id -> [(engine, pos, cum_value)]
    sem_engines = defaultdict(set)
    sem_unsafe = defaultdict(bool)
    engine_pos = defaultdict(int)
    cum = defaultdict(int)
    for bb in nc.main_func.blocks:
        for inst in bb.instructions:
            eng = inst.engine
            pos = engine_pos[eng]
            engine_pos[eng] = pos + 1
            si = inst.sync_info
            if si is None or not si.on_update:
                continue
            is_dma = isinstance(inst, bass_rust.InstDMACopy)
            for u in si.on_update:
                if u.sync_type != "semaphore" or u.update_mode != "sem-inc":
                    sem_unsafe[u.id] = True
                    continue
                cum[u.id] += u.update_value
                sem_updates[u.id].append((eng, pos, cum[u.id]))
                sem_engines[u.id].add(eng)
                if is_dma:
                    sem_unsafe[u.id] = True

    def satisfier(w):
        """(engine, queue_pos) of the update that first satisfies wait w, or
        None when the sem is DMA-completed, multi-queue, or unanalyzable."""
        if w.sync_type != "semaphore" or w.wait_mode != "sem-ge-imm":
            return None
        if sem_unsafe[w.id] or len(sem_engines[w.id]) != 1:
            return None
        for eng, pos, val in sem_updates[w.id]:
            if val >= w.wait_value:
                return (eng, pos)
        return None

    pruned = 0
    engine_pos = defaultdict(int)
    for bb in nc.main_func.blocks:
        for inst in bb.instructions:
            own = (inst.engine, engine_pos[inst.engine])
            engine_pos[inst.engine] = own[1] + 1
            si = inst.sync_info
            waits = list(si.on_wait) if (si is not None and si.on_wait) else []
            if not waits:
                continue
            sats = [satisfier(w) for w in waits]
            keep = []
            for i, (w, s) in enumerate(zip(waits, sats)):
                # implied by the instruction's own in-order engine: the
                # satisfying update was posted by an earlier instruction on
                # this queue, which has completed by the time we dispatch
                implied = s is not None and s[0] == own[0] and s[1] < own[1]
                # implied by another wait satisfied later on the same queue
                implied = implied or (
                    s is not None
                    and any(
                        j != i
                        and sj is not None
                        and sj[0] == s[0]
                        and (sj[1] > s[1] or (sj[1] == s[1] and j < i))
                        for j, sj in enumerate(sats)
                    )
                )
                if implied:
                    pruned += 1
                else:
                    keep.append(w)
            if len(keep) < len(waits):
                inst.sync_info = bass_rust.SyncInfo(
                    on_wait=keep, on_update=list(si.on_update or [])
                )
    return pruned


def _bcast_ap(dram_ap, parts):
    """Partition-stride-0 DMA source view of a 1-D DRAM tensor: [n] -> [parts, n]."""
    return bass.AP(
        tensor=dram_ap.tensor,
        offset=dram_ap.offset,
        ap=[[0, parts]] + list(dram_ap.ap),
    )


def build_nc(split_waits=True, loop_n=None, unroll=1):
    nc = bass.Bass(target_bir_lowering=False)

    xT_e = nc.declare_dram_parameter("xT", [D, P], BF, isOutput=False)
    wqk_e = nc.declare_dram_parameter("wqk", [D, 2 * D], BF, isOutput=False)
    wv_e = nc.declare_dram_parameter("wv", [D, D], BF, isOutput=False)
    wph_e = nc.declare_dram_parameter("wph", [H // 2, 128, D], BF, isOutput=False)
    bqk_e = nc.declare_dram_parameter("bqk", [128, 2 * D // 128], F32, isOutput=False)
    bv_e = nc.declare_dram_parameter("bv", [D], F32, isOutput=False)
    bp_e = nc.declare_dram_parameter("bp", [D], F32, isOutput=False)
    id_e = nc.declare_dram_parameter("ident", [128, 128], BF, isOutput=False)
    sel_e = nc.declare_dram_parameter("selmat", [H, H * HD], BF, isOutput=False)
    out_e = nc.declare_dram_parameter("out", [P, D], BF, isOutput=True)
    wa_e = nc.declare_dram_parameter("wa", [P, D], BF, isOutput=True)

    EXP = mybir.ActivationFunctionType.Exp

    with tile.TileContext(nc) as tc, ExitStack() as ctx:
        if loop_n is not None:
            ctx.enter_context(tc.For_i(0, loop_n, 1))
        const = ctx.enter_context(tc.tile_pool(name="const", bufs=1))
        qkp = ctx.enter_context(tc.tile_pool(name="qkp", bufs=1))
        vxp = ctx.enter_context(tc.tile_pool(name="vxp", bufs=1))
        wtp = ctx.enter_context(tc.tile_pool(name="wtp", bufs=1))
        ptp = ctx.enter_context(tc.tile_pool(name="ptp", bufs=28))
        stgp = ctx.enter_context(tc.tile_pool(name="stgp", bufs=1))
        outp = ctx.enter_context(tc.tile_pool(name="outp", bufs=3))
        # PSUM budget (8 banks): psS = 2 x [128,1024] S tiles (4 banks),
        # psAV = 2 x [128,512] (AV accumulation groups + upfront qkT units,
        # 2 banks), psX = 2 x [128,512] filler/psr/proj utility (2 banks).
        psS = ctx.enter_context(tc.tile_pool(name="psS", bufs=2, space="PSUM"))
        psAV = ctx.enter_context(tc.tile_pool(name="psAV", bufs=2, space="PSUM"))
        psX = ctx.enter_context(tc.tile_pool(name="psX", bufs=2, space="PSUM"))

        for _it in range(unroll):
            # ---- constant loads --------------------------------------------
            xT = [const.tile([128, P], BF, tag=f"xT{k}", name=f"xT{k}") for k in range(KT)]
            wqk = [const.tile([128, 2 * D], BF, tag=f"wqk{k}", name=f"wqk{k}") for k in range(KT)]
            wv = [const.tile([128, D], BF, tag=f"wv{k}", name=f"wv{k}") for k in range(KT)]
            wp = [const.tile([128, D], BF, tag=f"wp{p}", name=f"wp{p}") for p in range(H // 2)]
            bqk = const.tile([128, 2 * D // 128], F32, tag="bqk", name="bqk")
            bvb = const.tile([128, D], F32, tag="bvb", name="bvb")
            bpb = const.tile([128, D], F32, tag="bpb", name="bpb")
            ident = const.tile([128, 128], BF, tag="ident", name="ident")
            selmat = const.tile([H, H * HD], BF, tag="selmat", name="selmat")

            # DGE rings are in-order with head-of-line blocking, so queues are
            # segregated by readiness: needed-early input loads on SP,
            # mid-kernel small transfers on Pool, output writebacks on Act.
            # The upfront qkT units only read wqk columns for m in {0, 6}, so
            # those slices load first and the bulk follows after the upfront
            # units are emitted — this halves the startup DMA critical path.
            for k in range(KT):
                nc.sync.dma_start(out=xT[k], in_=xT_e[k * 128 : (k + 1) * 128, :])
                for m in (0, 6):
                    nc.sync.dma_start(
                        out=wqk[k][:, m * 128 : (m + 1) * 128],
                        in_=wqk_e[k * 128 : (k + 1) * 128, m * 128 : (m + 1) * 128],
                    )
                nc.gpsimd.dma_start(out=wv[k], in_=wv_e[k * 128 : (k + 1) * 128, :])
            nc.gpsimd.dma_start(out=bqk, in_=bqk_e[:])
            nc.gpsimd.dma_start(out=bvb, in_=_bcast_ap(bv_e[:], 128))
            nc.gpsimd.dma_start(out=selmat, in_=sel_e[:])
            for p in range(H // 2):
                nc.gpsimd.dma_start(out=wp[p], in_=wph_e[p])
            nc.gpsimd.dma_start(out=bpb, in_=_bcast_ap(bp_e[:], 128))
            nc.gpsimd.dma_start(out=ident, in_=id_e[:])

            # ---- phase 1: qT / kT = (w_qk)^T @ x^T  [feature-major] --------
            # qkT[m] rows = features m*128..; m 0..5 -> q, 6..11 -> k.
            # Pair pr's S needs tiles {pr, 6+pr}; the remaining tiles are
            # emitted via the filler stream inside the attention windows.
            qkT = [qkp.tile([128, P], BF, tag=f"qkT{m}", name=f"qkT{m}") for m in range(2 * D // 128)]

            def qkT_closures(ms, pool=None):
                """One closure per matmul; the last of each (m, j) unit also
                emits the DVE bias-add evacuation into the qkT tile."""
                cls = []
                for m in ms:
                    for j in range(2):
                        st = {}

                        def mk(k, m=m, j=j, st=st):
                            def go():
                                if k == 0:
                                    st["ps"] = (pool or psX).tile(
                                        [128, 512], F32,
                                        tag="px" if pool is None else "av", name="px")
                                nc.tensor.matmul(
                                    st["ps"],
                                    lhsT=wqk[k][:, m * 128 : (m + 1) * 128],
                                    rhs=xT[k][:, j * 512 : (j + 1) * 512],
                                    start=(k == 0),
                                    stop=(k == KT - 1),
                                )
                                if k == KT - 1:
                                    nc.vector.tensor_scalar_add(
                                        qkT[m][:, j * 512 : (j + 1) * 512],
                                        st["ps"],
                                        bqk[:, m : m + 1],
                                    )
                            return go

                        cls.extend(mk(k) for k in range(KT))
                return cls

            # upfront units (pair 0's q/k tiles) draw from psAV, whose last
            # prior use (the previous iteration's AV(5) groups) clears early
            # in that iteration's tail
            for cl in qkT_closures([0, 6], pool=psAV):
                cl()

            # ---- phase 2: v natural [seq-major] with ones column -----------
            # vext[p][:, h, 0:64] = v_h rows p*128..; vext[p][:, h, 64] = 1.0
            # Emitted as filler closures inside window 0 (memsets upfront).
            vext = [vxp.tile([128, H, HD + 1], BF, tag=f"vext{p}", name=f"vext{p}") for p in range(QT)]
            for p in range(QT):
                nc.vector.memset(vext[p][:, :, HD : HD + 1], 1.0)

            def vext_closures():
                cls = []
                for p in range(QT):
                    for (c0, cw) in ((0, 512), (512, 256)):
                        st = {}

                        def mk(k, p=p, c0=c0, cw=cw, st=st):
                            def go():
                                if k == 0:
                                    st["ps"] = psX.tile([128, 512], F32, tag="px", name="px")
                                nc.tensor.matmul(
                                    st["ps"][:, :cw],
                                    lhsT=xT[k][:, p * 128 : (p + 1) * 128],
                                    rhs=wv[k][:, c0 : c0 + cw],
                                    start=(k == 0),
                                    stop=(k == KT - 1),
                                )
                                if k == KT - 1:
                                    nh = cw // HD
                                    nc.vector.tensor_add(
                                        vext[p][:, c0 // HD : c0 // HD + nh, 0:HD],
                                        st["ps"][:, :cw].rearrange("p (h d) -> p h d", d=HD),
                                        bvb[:, c0 : c0 + cw].rearrange("p (h d) -> p h d", d=HD),
                                    )
                            return go

                        cls.extend(mk(k) for k in range(KT))
                return cls

            # ---- phase 3: six per-pair windows -----------------------------
            # Window p: per k-tile, S^T for heads (2p, 2p+1) into two
            # [128,1024] PSUM tiles (even head stationary at PE rows 0-63,
            # odd at 64-127 -> adjacent matmuls in distinct row-groups), one
            # [128,1024] exp each. pt tiles persist in SBUF; AV for pair p
            # runs as filler in window p+1, one (head, j) group at a time.
            waTp = [wtp.tile([128, P], BF, tag=f"waTp{p}", name=f"waTp{p}") for p in range(H // 2)]
            dens12 = stgp.tile([H, P], BF, tag="dens12", name="dens12")
            recip12 = stgp.tile([H, P], F32, tag="recip12", name="recip12")
            recip12b = stgp.tile([H, P], BF, tag="recip12b", name="recip12b")
            nc.vector.memset(dens12, 1.0)
            pt_tiles = {}   # (h, kt) -> SBUF [128, 1024] bf16
            stg_tiles = {}  # (h, j) -> SBUF [65, 512] bf16

            def av_closures(pr):
                """AV for pair pr: per (head, j-half) one 8-matmul PSUM
                accumulation group, evacuated to stg; the denominator row
                rides partition 64 and is copied into dens12 by DMA."""
                cls = []
                for h in (2 * pr, 2 * pr + 1):
                    for j in range(2):
                        st = {}

                        def mk(kt, h=h, j=j, st=st):
                            def go():
                                if kt == 0:
                                    st["ps"] = psAV.tile([128, 512], F32, tag="av", name="av")
                                nc.tensor.matmul(
                                    st["ps"][: HD + 1, :],
                                    lhsT=vext[kt][:, h, :],
                                    rhs=pt_tiles[(h, kt)][:, j * 512 : (j + 1) * 512],
                                    start=(kt == 0),
                                    stop=(kt == QT - 1),
                                )
                                if kt == QT - 1:
                                    stg = stgp.tile(
                                        [HD + 1, 512], BF, tag=f"stg{h}j{j}", name=f"stg{h}j{j}"
                                    )
                                    nc.vector.tensor_copy(stg, st["ps"][: HD + 1, :])
                                    nc.sync.dma_start(
                                        out=dens12[h : h + 1, j * 512 : (j + 1) * 512],
                                        in_=stg[HD : HD + 1, :],
                                    )
                                    stg_tiles[(h, j)] = stg
                            return go

                        cls.extend(mk(kt) for kt in range(QT))
                return cls

            def recip_batch():
                """Softmax reciprocals on the idle VectorEngine (keeps the
                ScalarEngine free for the exp stream)."""
                nc.vector.reciprocal(recip12[0:H, :], dens12[0:H, :])
                nc.vector.tensor_copy(recip12b[0:H, :], recip12[0:H, :])

            def norm_closures(heads):
                """Per (h, j): recip broadcast via selector matmul, then DVE
                multiply of the staged AV rows; odd heads merge into the pair
                tile's upper partitions by DMA."""
                cls = []
                for h in heads:
                    for j in range(2):
                        def go(h=h, j=j):
                            psr = psX.tile([HD, 512], F32, tag="px", name="psr")
                            nc.tensor.matmul(
                                psr,
                                lhsT=selmat[:, h * HD : (h + 1) * HD],
                                rhs=recip12b[0:H, j * 512 : (j + 1) * 512],
                                start=True,
                                stop=True,
                            )
                            sl = slice(j * 512, (j + 1) * 512)
                            if h % 2 == 0:
                                nc.vector.tensor_mul(
                                    waTp[h // 2][0:HD, sl], stg_tiles[(h, j)][0:HD, :], psr
                                )
                            else:
                                wt = outp.tile([HD, 512], BF, tag="wtmp", name="wtmp")
                                nc.vector.tensor_mul(wt, stg_tiles[(h, j)][0:HD, :], psr)
                                nc.sync.dma_start(out=waTp[h // 2][HD:128, sl], in_=wt)
                        cls.append(go)
                return cls

            filler = deque()
            filler.extend(vext_closures())
            # per-window pop quota per k-tile: window 0 drains vext (96) +
            # pair-1 qkT (24); windows 1-4 drain AV(p-1) (32) + qkT(p+1)
            # (24); window 5 drains AV(4) (32) + norm heads 0-7 (16).
            POPS = (15, 7, 7, 7, 7, 6)
            for pr in range(H // 2):
                heads = (2 * pr, 2 * pr + 1)
                if pr >= 1:
                    # AV feeds the exp pipeline's pt-buffer recycling (hard
                    # dependency) so it queues ahead of the next pair's qkT
                    filler.extend(av_closures(pr - 1))
                if pr + 1 < H // 2:
                    filler.extend(qkT_closures([pr + 1, 6 + pr + 1]))
                if pr == 5:
                    # recip batch 1 (heads 0-7) is emitted mid-window-5 below;
                    # norm for heads 0-7 follows as late window-5 filler
                    filler.extend(norm_closures(range(0, 8)))
                for kt in range(QT):
                    pss = {}
                    for h in heads:
                        pss[h] = psS.tile([128, 1024], F32, tag="ss", name="ss")
                    for j in range(2):
                        for h in heads:
                            base = (h % 2) * 64
                            nc.tensor.matmul(
                                pss[h][:, j * 512 : (j + 1) * 512],
                                lhsT=qkT[6 + pr][base : base + 64, kt * 128 : (kt + 1) * 128],
                                rhs=qkT[pr][base : base + 64, j * 512 : (j + 1) * 512],
                                start=True,
                                stop=True,
                            )
                    for h in heads:
                        pt = ptp.tile([128, 1024], BF, tag="pt", name="pt")
                        nc.scalar.activation(pt, pss[h], EXP, scale=SCALE)
                        pt_tiles[(h, kt)] = pt
                    if pr == 5 and kt == 3:
                        # heads 0-7 denominators are complete (AV(3) ran in
                        # window 4); batch the reciprocals now so norm(0-2)
                        # can run as this window's late filler
                        recip_batch()
                    # filler pacing: drain this window's quota evenly
                    for _ in range(POPS[pr]):
                        if filler:
                            filler.popleft()()

            # ---- tail: AV(5), late norms, wa transposes fused with proj ----
            for cl in av_closures(5):
                cl()
            while filler:
                filler.popleft()()
            recip_batch()
            for cl in norm_closures(range(8, H)):
                cl()

            def emit_transposes(qt, prs, psw):
                for p in prs:
                    nc.tensor.matmul(
                        psw[:, p * 128 : (p + 1) * 128],
                        lhsT=waTp[p][:, qt * 128 : (qt + 1) * 128],
                        rhs=ident,
                        start=True,
                        stop=True,
                    )

            def emit_proj(qt, ps, c0, cw, prs, start, stop):
                for i, p in enumerate(prs):
                    nc.tensor.matmul(
                        ps[:, c0 : c0 + cw],
                        lhsT=waTp[p][:, qt * 128 : (qt + 1) * 128],
                        rhs=wp[p][:, c0 : c0 + cw],
                        start=start and i == 0,
                        stop=stop and i == len(prs) - 1,
                        skip_group_check=True,
                    )

            def emit_evacs(qt, psw, ps):
                wa_sb = outp.tile([128, D], BF, tag="wa_sb", name="wa_sb")
                nc.scalar.copy(wa_sb, psw[:, :D])
                nc.scalar.dma_start(out=wa_e[qt * 128 : (qt + 1) * 128, :], in_=wa_sb)
                out_sb = outp.tile([128, D], BF, tag="out_sb", name="out_sb")
                nc.vector.tensor_add(out_sb, ps[:, :D], bpb)
                nc.scalar.dma_start(out=out_e[qt * 128 : (qt + 1) * 128, :], in_=out_sb)

            # per q-tile: one [128,1024] psS tile holds the wa transpose
            # (cols 0-767), a second holds the proj accumulation (cols 0-767)
            for qt in range(QT):
                psw = psS.tile([128, 1024], F32, tag="ss", name="psw")
                emit_transposes(qt, range(H // 2), psw)
                ps = psS.tile([128, 1024], F32, tag="ss", name="psp")
                for (c0, cw) in ((0, 512), (512, 256)):
                    emit_proj(qt, ps, c0, cw, range(H // 2), True, True)
                emit_evacs(qt, psw, ps)

    if split_waits:
        _prune_implied_waits(nc)
        _split_excess_waits(nc)
    return nc


def make_in_maps(x, w_qkv, b_qkv, w_proj, b_proj):
    """Host-side shard prep: batch element b -> core b; weights replicated."""
    xf = np.asarray(x, dtype=np.float32)
    wqkv = np.asarray(w_qkv, dtype=np.float32)
    bqkv = np.asarray(b_qkv, dtype=np.float32)
    wproj = np.asarray(w_proj, dtype=np.float32)
    bproj = np.asarray(b_proj, dtype=np.float32)

    wqk = np.ascontiguousarray(wqkv[:, : 2 * D]).astype(NP_BF16)
    wv = np.ascontiguousarray(wqkv[:, 2 * D :]).astype(NP_BF16)
    wph = np.ascontiguousarray(wproj.reshape(H // 2, 128, D)).astype(NP_BF16)
    bqk = np.ascontiguousarray(bqkv[: 2 * D].reshape(2 * D // 128, 128).T)
    bv = np.ascontiguousarray(bqkv[2 * D :])
    ident = np.eye(128, dtype=np.float32).astype(NP_BF16)
    selmat = np.kron(np.eye(H, dtype=np.float32), np.ones((1, HD), np.float32)).astype(NP_BF16)

    in_maps = []
    for b in range(N_CORES):
        in_maps.append(
            {
                "xT": np.ascontiguousarray(xf[b].T).astype(NP_BF16),
                "wqk": wqk,
                "wv": wv,
                "wph": wph,
                "bqk": bqk,
                "bv": bv,
                "bp": bproj,
                "ident": ident,
                "selmat": selmat,
            }
        )
    return in_maps


_CACHE = {}


def _get_nc():
    if "nc" not in _CACHE:
        _CACHE["nc"] = build_nc()
    return _CACHE["nc"]


def run_once(in_maps, nc=None):
    """One 8-core execution via the PJRT redirect path (fresh jit per call;
    NEFF comes from the neuron compile cache after the first call)."""
    if nc is None:
        nc = _get_nc()
    return bass2jax.run_bass_via_pjrt(nc, in_maps, n_cores=N_CORES)


def kernel(x, w_qkv, b_qkv, w_proj, b_proj):
    in_maps = make_in_maps(x, w_qkv, b_qkv, w_proj, b_proj)
    results = run_once(in_maps)
    out = np.stack([results[b]["out"] for b in range(N_CORES)]).astype(np.float32)
    wa = np.stack([results[b]["wa"] for b in range(N_CORES)]).astype(np.float32)
    return (out, wa)


# revision 39
# speedup vs baseline: 1.1196x; 1.0282x over previous
"""Multi-head attention (B=8, P=1024, D=768, H=12) on 8 TRN2 NeuronCores.

Strategy: pure data parallelism — batch element b runs on core b (no
collectives). Host pre-transposes x and casts operands to bf16; each core
computes QK^T/softmax/AV/proj for its batch element with all matmuls on the
TensorEngine (bf16, fp32 PSUM accumulation), exp on the ScalarEngine, and
evacuations/normalization on the VectorEngine.

Schedule (v2): the attention loop is organized as 6 per-pair "windows" paced
by the ScalarEngine exp stream. Per k-tile, the two heads' S^T matmuls write
two [128,1024] PSUM tiles (even head at PE rows 0-63, odd at 64-127, so
adjacent matmuls occupy distinct row-groups and overlap in the array), each
evacuated by ONE [128,1024] exp (wider activations amortize the ~185ns
SBUF-ack overhead). exp outputs (pt) are held in SBUF for a full window; the
AV matmuls for pair p run as PE filler inside pair p+1's window, one
(head, j-half) accumulation group at a time into a single PSUM bank — this
frees enough PSUM for the double-size S tiles. Remaining PE filler: the next
pair's qkT feature tiles, the v-projection (window 0), and the per-head
normalization broadcasts. Softmax reciprocals run on the idle VectorEngine
(InstReciprocal) instead of the ScalarEngine ln/exp chain so they don't
steal exp throughput. The tail does per-q-tile wa transposes fused with the
output projection, as before.

Self-contained: builds + compiles the Bass kernel on first call, runs via
PJRT (axon) across cores 0-7, and reassembles full outputs. Returns the
tuple (out, weighted_avg), matching the reference.
"""

import numpy as np
from collections import deque
from contextlib import ExitStack

import bass_rust
import concourse.bass as bass
import concourse.tile as tile
from concourse import mybir
from concourse import bass2jax

B, P, D = 8, 1024, 768
H = 12
HD = D // H            # 64
SCALE = HD ** -0.5     # 0.125
N_CORES = 8
KT = D // 128          # 6 contraction tiles over d
QT = P // 128          # 8 tiles over sequence
BF = mybir.dt.bfloat16
F32 = mybir.dt.float32
NP_BF16 = np.dtype(mybir.dt.np(BF))

IN_NAMES = ["xT", "wqk", "wv", "wph", "bqk", "bv", "bp", "ident", "selmat"]
OUT_NAMES = ["out", "wa"]


def _split_excess_waits(nc, max_waits=1):
    """This container's walrus build rejects instructions carrying more than
    one sync wait. Hoist excess waits onto same-engine no-ops inserted just
    before the overloaded instruction (engine queues execute in order, so
    wait-for-all-before-exec semantics are preserved)."""
    ctr = 0
    for bb in nc.main_func.blocks:
        newlist = []
        dirty = False
        for inst in bb.instructions:
            si = inst.sync_info
            waits = list(si.on_wait) if (si is not None and si.on_wait) else []
            if len(waits) > max_waits:
                excess, keep = waits[:-max_waits], waits[-max_waits:]
                for i in range(0, len(excess), max_waits):
                    chunk = excess[i : i + max_waits]
                    nop = bass_rust.InstNoOp(name=f"WSPILL-{ctr}")
                    ctr += 1
                    nop.engine = inst.engine
                    nop.sync_info = bass_rust.SyncInfo(on_wait=chunk, on_update=[])
                    newlist.append(nop)
                inst.sync_info = bass_rust.SyncInfo(
                    on_wait=keep, on_update=list(si.on_update or [])
                )
                dirty = True
            newlist.append(inst)
        if dirty:
            bb.instructions = newlist
    return ctr


def _prune_implied_waits(nc):
    """Drop transitively-implied waits: when an instruction carries two waits
    whose satisfying semaphore updates are both posted by the SAME in-order
    engine queue (non-DMA, sem-inc only), the wait satisfied earlier in that
    queue is implied by the later one — in-order engines post updates in
    order. This removes most multi-wait spills so the sequencers don't
    execute a wait no-op before every instruction."""
    from collections import defaultdict

    sem_updates = defaultdict(list)  # sem id -> [(engine, pos, cum_value)]
    sem_engines = defaultdict(set)
    sem_unsafe = defaultdict(bool)
    engine_pos = defaultdict(int)
    cum = defaultdict(int)
    for bb in nc.main_func.blocks:
        for inst in bb.instructions:
            eng = inst.engine
            pos = engine_pos[eng]
            engine_pos[eng] = pos + 1
            si = inst.sync_info
            if si is None or not si.on_update:
                continue
            is_dma = isinstance(inst, bass_rust.InstDMACopy)
            for u in si.on_update:
                if u.sync_type != "semaphore" or u.update_mode != "sem-inc":
                    sem_unsafe[u.id] = True
                    continue
                cum[u.id] += u.update_value
                sem_updates[u.id].append((eng, pos, cum[u.id]))
                sem_engines[u.id].add(eng)
                if is_dma:
                    sem_unsafe[u.id] = True

    def satisfier(w):
        """(engine, queue_pos) of the update that first satisfies wait w, or
        None when the sem is DMA-completed, multi-queue, or unanalyzable."""
        if w.sync_type != "semaphore" or w.wait_mode != "sem-ge-imm":
            return None
        if sem_unsafe[w.id] or len(sem_engines[w.id]) != 1:
            return None
        for eng, pos, val in sem_updates[w.id]:
            if val >= w.wait_value:
                return (eng, pos)
        return None

    pruned = 0
    engine_pos = defaultdict(int)
    for bb in nc.main_func.blocks:
        for inst in bb.instructions:
            own = (inst.engine, engine_pos[inst.engine])
            engine_pos[inst.engine] = own[1] + 1
            si = inst.sync_info
            waits = list(si.on_wait) if (si is not None and si.on_wait) else []
            if not waits:
                continue
            sats = [satisfier(w) for w in waits]
            keep = []
            for i, (w, s) in enumerate(zip(waits, sats)):
                # implied by the instruction's own in-order engine: the
                # satisfying update was posted by an earlier instruction on
                # this queue, which has completed by the time we dispatch
                implied = s is not None and s[0] == own[0] and s[1] < own[1]
                # implied by another wait satisfied later on the same queue
                implied = implied or (
                    s is not None
                    and any(
                        j != i
                        and sj is not None
                        and sj[0] == s[0]
                        and (sj[1] > s[1] or (sj[1] == s[1] and j < i))
                        for j, sj in enumerate(sats)
                    )
                )
                if implied:
                    pruned += 1
                else:
                    keep.append(w)
            if len(keep) < len(waits):
                inst.sync_info = bass_rust.SyncInfo(
                    on_wait=keep, on_update=list(si.on_update or [])
                )
    return pruned


def _bcast_ap(dram_ap, parts):
    """Partition-stride-0 DMA source view of a 1-D DRAM tensor: [n] -> [parts, n]."""
    return bass.AP(
        tensor=dram_ap.tensor,
        offset=dram_ap.offset,
        ap=[[0, parts]] + list(dram_ap.ap),
    )


def build_nc(split_waits=True, loop_n=None, unroll=1):
    nc = bass.Bass(target_bir_lowering=False)

    xT_e = nc.declare_dram_parameter("xT", [D, P], BF, isOutput=False)
    wqk_e = nc.declare_dram_parameter("wqk", [D, 2 * D], BF, isOutput=False)
    wv_e = nc.declare_dram_parameter("wv", [D, D], BF, isOutput=False)
    wph_e = nc.declare_dram_parameter("wph", [H // 2, 128, D], BF, isOutput=False)
    bqk_e = nc.declare_dram_parameter("bqk", [128, 2 * D // 128], F32, isOutput=False)
    bv_e = nc.declare_dram_parameter("bv", [D], F32, isOutput=False)
    bp_e = nc.declare_dram_parameter("bp", [D], F32, isOutput=False)
    id_e = nc.declare_dram_parameter("ident", [128, 128], BF, isOutput=False)
    sel_e = nc.declare_dram_parameter("selmat", [H, H * HD], BF, isOutput=False)
    out_e = nc.declare_dram_parameter("out", [P, D], BF, isOutput=True)
    wa_e = nc.declare_dram_parameter("wa", [P, D], BF, isOutput=True)

    EXP = mybir.ActivationFunctionType.Exp

    with tile.TileContext(nc) as tc, ExitStack() as ctx:
        if loop_n is not None:
            ctx.enter_context(tc.For_i(0, loop_n, 1))
        const = ctx.enter_context(tc.tile_pool(name="const", bufs=1))
        qkp = ctx.enter_context(tc.tile_pool(name="qkp", bufs=1))
        vxp = ctx.enter_context(tc.tile_pool(name="vxp", bufs=1))
        wtp = ctx.enter_context(tc.tile_pool(name="wtp", bufs=1))
        ptp = ctx.enter_context(tc.tile_pool(name="ptp", bufs=26))
        stgp = ctx.enter_context(tc.tile_pool(name="stgp", bufs=1))
        outp = ctx.enter_context(tc.tile_pool(name="outp", bufs=2))
        # PSUM budget (8 banks): psS = 2 x [128,1024] S tiles (4 banks),
        # psAV = 2 x [128,512] (AV accumulation groups + upfront qkT units,
        # 2 banks), psX = 2 x [128,512] filler/psr/proj utility (2 banks).
        psS = ctx.enter_context(tc.tile_pool(name="psS", bufs=2, space="PSUM"))
        psAV = ctx.enter_context(tc.tile_pool(name="psAV", bufs=2, space="PSUM"))
        psX = ctx.enter_context(tc.tile_pool(name="psX", bufs=2, space="PSUM"))

        def alloc_xT_wqk():
            """xT + the wqk feature slices needed by the upfront qkT units.
            xT is double-buffered so a later iteration's loads can issue while
            the previous buffers are still being read."""
            xT = [const.tile([128, P], BF, tag=f"xT{k}", name=f"xT{k}", bufs=2) for k in range(KT)]
            wqk = [const.tile([128, 2 * D], BF, tag=f"wqk{k}", name=f"wqk{k}") for k in range(KT)]
            for k in range(KT):
                nc.sync.dma_start(out=xT[k], in_=xT_e[k * 128 : (k + 1) * 128, :])
                for m in (0, 6):
                    nc.sync.dma_start(
                        out=wqk[k][:, m * 128 : (m + 1) * 128],
                        in_=wqk_e[k * 128 : (k + 1) * 128, m * 128 : (m + 1) * 128],
                    )
            return xT, wqk

        pending = {}
        # qkT tiles rotate across pairs AND iterations: entries created by a
        # tail's cross-iteration upfront units must survive into the next
        # iteration body, so the dict lives outside the unroll loop
        qkT = {}

        for _it in range(unroll):
            # ---- constant loads --------------------------------------------
            if "xT" in pending:
                # loads + upfront qkT units were already emitted in the
                # previous iteration's tail (cross-iteration pipelining)
                xT, wqk = pending.pop("xT"), pending.pop("wqk")
                upfront_done = True
            else:
                xT, wqk = alloc_xT_wqk()
                upfront_done = False
            wv = [const.tile([128, D], BF, tag=f"wv{k}", name=f"wv{k}") for k in range(KT)]
            wp = [const.tile([128, D], BF, tag=f"wp{p}", name=f"wp{p}") for p in range(H // 2)]
            bqk = const.tile([128, 2 * D // 128], F32, tag="bqk", name="bqk")
            bvb = const.tile([128, D], F32, tag="bvb", name="bvb")
            bpb = const.tile([128, D], F32, tag="bpb", name="bpb")
            ident = const.tile([128, 128], BF, tag="ident", name="ident")
            selmat = const.tile([H, H * HD], BF, tag="selmat", name="selmat")

            # DGE rings are in-order with head-of-line blocking, so queues are
            # segregated by readiness: needed-early input loads on SP,
            # mid-kernel small transfers on Pool, output writebacks on Act.
            # The upfront qkT units only read wqk columns for m in {0, 6}
            # (loaded in alloc_xT_wqk); the bulk follows after the upfront
            # units are emitted — this halves the startup DMA critical path.
            for k in range(KT):
                nc.gpsimd.dma_start(out=wv[k], in_=wv_e[k * 128 : (k + 1) * 128, :])
            nc.gpsimd.dma_start(out=bqk, in_=bqk_e[:])
            nc.gpsimd.dma_start(out=bvb, in_=_bcast_ap(bv_e[:], 128))
            nc.gpsimd.dma_start(out=selmat, in_=sel_e[:])
            for p in range(H // 2):
                nc.gpsimd.dma_start(out=wp[p], in_=wph_e[p])
            nc.gpsimd.dma_start(out=bpb, in_=_bcast_ap(bp_e[:], 128))
            nc.gpsimd.dma_start(out=ident, in_=id_e[:])

            # ---- phase 1: qT / kT = (w_qk)^T @ x^T  [feature-major] --------
            # qkT[m] rows = features m*128..; m 0..5 -> q, 6..11 -> k.
            # Pair pr's S needs tiles {pr, 6+pr}; the remaining tiles are
            # emitted via the filler stream inside the attention windows.
            def qkT_closures(ms, pool=None, xT_=None, wqk_=None):
                """One closure per matmul; the last of each (m, j) unit also
                emits the DVE bias-add evacuation into the qkT tile. xT_/wqk_
                override the input tiles (used when emitting the NEXT
                iteration's upfront units inside this iteration's tail)."""
                xTl = xT_ if xT_ is not None else xT
                wqkl = wqk_ if wqk_ is not None else wqk
                cls = []
                for m in ms:
                    for j in range(2):
                        st = {}

                        def mk(k, m=m, j=j, st=st):
                            def go():
                                if k == 0 and j == 0:
                                    qkT[m] = qkp.tile(
                                        [128, P], BF,
                                        tag=f"qkT{'q' if m < 6 else 'k'}{m % 2}",
                                        name=f"qkT{m}", bufs=2)
                                if k == 0:
                                    st["ps"] = (pool or psX).tile(
                                        [128, 512], F32,
                                        tag="px" if pool is None else "av", name="px")
                                nc.tensor.matmul(
                                    st["ps"],
                                    lhsT=wqkl[k][:, m * 128 : (m + 1) * 128],
                                    rhs=xTl[k][:, j * 512 : (j + 1) * 512],
                                    start=(k == 0),
                                    stop=(k == KT - 1),
                                )
                                if k == KT - 1:
                                    nc.vector.tensor_scalar_add(
                                        qkT[m][:, j * 512 : (j + 1) * 512],
                                        st["ps"],
                                        bqk[:, m : m + 1],
                                    )
                            return go

                        cls.extend(mk(k) for k in range(KT))
                return cls

            # upfront units (pair 0's q/k tiles) draw from psAV, whose last
            # prior use (the previous iteration's AV(5) groups) clears early
            # in that iteration's tail
            if not upfront_done:
                for cl in qkT_closures([0, 6], pool=psAV):
                    cl()
            # bulk of the q/k weights (features m not in {0, 6}), consumed by
            # the qkT filler units from window 0 onward
            for k in range(KT):
                nc.sync.dma_start(
                    out=wqk[k][:, 128:768],
                    in_=wqk_e[k * 128 : (k + 1) * 128, 128:768],
                )
                nc.sync.dma_start(
                    out=wqk[k][:, 896 : 2 * D],
                    in_=wqk_e[k * 128 : (k + 1) * 128, 896 : 2 * D],
                )

            # ---- phase 2: v natural [seq-major] with ones column -----------
            # vext[p][:, h, 0:64] = v_h rows p*128..; vext[p][:, h, 64] = 1.0
            # Emitted as filler closures inside window 0 (memsets upfront).
            vext = [vxp.tile([128, H, HD + 1], BF, tag=f"vext{p}", name=f"vext{p}") for p in range(QT)]
            for p in range(QT):
                nc.vector.memset(vext[p][:, :, HD : HD + 1], 1.0)

            def vext_closures():
                cls = []
                for p in range(QT):
                    for (c0, cw) in ((0, 512), (512, 256)):
                        st = {}

                        def mk(k, p=p, c0=c0, cw=cw, st=st):
                            def go():
                                if k == 0:
                                    st["ps"] = psX.tile([128, 512], F32, tag="px", name="px")
                                nc.tensor.matmul(
                                    st["ps"][:, :cw],
                                    lhsT=xT[k][:, p * 128 : (p + 1) * 128],
                                    rhs=wv[k][:, c0 : c0 + cw],
                                    start=(k == 0),
                                    stop=(k == KT - 1),
                                )
                                if k == KT - 1:
                                    nh = cw // HD
                                    nc.vector.tensor_add(
                                        vext[p][:, c0 // HD : c0 // HD + nh, 0:HD],
                                        st["ps"][:, :cw].rearrange("p (h d) -> p h d", d=HD),
                                        bvb[:, c0 : c0 + cw].rearrange("p (h d) -> p h d", d=HD),
                                    )
                            return go

                        cls.extend(mk(k) for k in range(KT))
                return cls

            # ---- phase 3: six per-pair windows -----------------------------
            # Window p: per k-tile, S^T for heads (2p, 2p+1) into two
            # [128,1024] PSUM tiles (even head stationary at PE rows 0-63,
            # odd at 64-127 -> adjacent matmuls in distinct row-groups), one
            # [128,1024] exp each. pt tiles persist in SBUF; AV for pair p
            # runs as filler in window p+1, one (head, j) group at a time.
            waTp = [wtp.tile([128, P], BF, tag=f"waTp{p}", name=f"waTp{p}") for p in range(H // 2)]
            dens12 = stgp.tile([H, P], F32, tag="dens12", name="dens12")
            recip12 = stgp.tile([H, P], F32, tag="recip12", name="recip12")
            recip12b = stgp.tile([H, P], BF, tag="recip12b", name="recip12b")
            nc.vector.memset(dens12, 1.0)
            # the norm selector matmuls contract all 12 recip12b rows (rows
            # for not-yet-normalized heads hit zero selector entries, but
            # must hold finite values, not uninitialized SBUF)
            nc.vector.memset(recip12b, 1.0)
            pt_tiles = {}   # (h, kt) -> SBUF [128, 1024] bf16
            stg_tiles = {}  # (h, j) -> SBUF [65, 512] bf16

            def av_closures(pr):
                """AV for pair pr: per (head, j-half) one 8-matmul PSUM
                accumulation group, evacuated to stg; the denominator row
                rides partition 64 and is copied into dens12 by DMA."""
                cls = []
                for h in (2 * pr, 2 * pr + 1):
                    for j in range(2):
                        st = {}

                        def mk(kt, h=h, j=j, st=st):
                            def go():
                                if kt == 0:
                                    st["ps"] = psAV.tile([128, 512], F32, tag="av", name="av")
                                nc.tensor.matmul(
                                    st["ps"][: HD + 1, :],
                                    lhsT=vext[kt][:, h, :],
                                    rhs=pt_tiles[(h, kt)][:, j * 512 : (j + 1) * 512],
                                    start=(kt == 0),
                                    stop=(kt == QT - 1),
                                )
                                if kt == QT - 1:
                                    # denominator row evacuated first (tiny
                                    # copy) so the reciprocal chain doesn't
                                    # wait on the full stg evacuation
                                    dstg = stgp.tile(
                                        [1, 512], F32, tag="den", name=f"den{h}j{j}", bufs=2
                                    )
                                    nc.vector.tensor_copy(dstg, st["ps"][HD : HD + 1, :])
                                    nc.sync.dma_start(
                                        out=dens12[h : h + 1, j * 512 : (j + 1) * 512],
                                        in_=dstg,
                                    )
                                    stg = stgp.tile(
                                        [HD, 512], BF, tag=f"stg{h}j{j}", name=f"stg{h}j{j}"
                                    )
                                    nc.vector.tensor_copy(stg, st["ps"][:HD, :])
                                    stg_tiles[(h, j)] = stg
                            return go

                        cls.extend(mk(kt) for kt in range(QT))
                return cls

            def recip_rows(a, b):
                """Softmax reciprocals on the idle VectorEngine (keeps the
                ScalarEngine free for the exp stream). Engine partition
                windows must be 32-aligned, so every batch recomputes all 12
                rows; rows whose denominators haven't landed yet hold
                memset/stale values and are only read by later batches'
                consumers after their real values arrive."""
                nc.vector.reciprocal(recip12[0:H, :], dens12[0:H, :])
                nc.vector.tensor_copy(recip12b[0:H, :], recip12[0:H, :])

            def norm_closures(heads, pool=None):
                """Per (h, j): recip broadcast via selector matmul, then DVE
                multiply of the staged AV rows; odd heads merge into the pair
                tile's upper partitions by DMA. The tail passes pool=psAV —
                psX is occupied by the open qt0 proj accumulators there."""
                cls = []
                for h in heads:
                    for j in range(2):
                        def go(h=h, j=j):
                            psr = (pool or psX).tile(
                                [HD, 512], F32,
                                tag="px" if pool is None else "av", name="psr")
                            nc.tensor.matmul(
                                psr,
                                lhsT=selmat[:, h * HD : (h + 1) * HD],
                                rhs=recip12b[0:H, j * 512 : (j + 1) * 512],
                                start=True,
                                stop=True,
                            )
                            sl = slice(j * 512, (j + 1) * 512)
                            if h % 2 == 0:
                                nc.vector.tensor_mul(
                                    waTp[h // 2][0:HD, sl], stg_tiles[(h, j)], psr
                                )
                            else:
                                wt = outp.tile([HD, 512], BF, tag="wtmp", name="wtmp")
                                nc.vector.tensor_mul(wt, stg_tiles[(h, j)], psr)
                                nc.sync.dma_start(out=waTp[h // 2][HD:128, sl], in_=wt)
                        cls.append(go)
                return cls

            filler = deque()
            filler.extend(vext_closures())
            # per-window pop quota per k-tile: window 0 drains vext (96) +
            # pair-1 qkT (24); windows 1-4 drain AV(p-1) (32) + qkT(p+1)
            # (24); window 5 drains AV(4) (32) + norm heads 0-7 (16).
            POPS = (15, 7, 7, 7, 7, 7)
            for pr in range(H // 2):
                heads = (2 * pr, 2 * pr + 1)
                if pr >= 1:
                    # AV feeds the exp pipeline's pt-buffer recycling (hard
                    # dependency) so it queues ahead of the next pair's qkT
                    filler.extend(av_closures(pr - 1))
                if pr + 1 < H // 2:
                    filler.extend(qkT_closures([pr + 1, 6 + pr + 1]))
                if pr == 5:
                    # recip batch 1 (heads 0-7) is emitted mid-window-5 below;
                    # norm for heads 0-7 follows as late window-5 filler, then
                    # pair-4 heads get their reciprocals + norm (their
                    # denominators complete once AV(4) drains early in this
                    # window), leaving only heads 10-11 for the tail chain
                    filler.extend(norm_closures(range(0, 8)))

                    def recip89():
                        recip_rows(8, 10)

                    filler.append(recip89)
                    filler.extend(norm_closures(range(8, 10)))
                for kt in range(QT):
                    pss = {}
                    for h in heads:
                        pss[h] = psS.tile([128, 1024], F32, tag="ss", name="ss")
                    for j in range(2):
                        for h in heads:
                            base = (h % 2) * 64
                            nc.tensor.matmul(
                                pss[h][:, j * 512 : (j + 1) * 512],
                                lhsT=qkT[6 + pr][base : base + 64, kt * 128 : (kt + 1) * 128],
                                rhs=qkT[pr][base : base + 64, j * 512 : (j + 1) * 512],
                                start=True,
                                stop=True,
                            )
                    for h in heads:
                        pt = ptp.tile([128, 1024], BF, tag="pt", name="pt")
                        nc.scalar.activation(pt, pss[h], EXP, scale=SCALE)
                        pt_tiles[(h, kt)] = pt
                    if pr == 5 and kt == 3:
                        # heads 0-7 denominators are complete (AV(3) ran in
                        # window 4); batch the reciprocals now so norm(0-7)
                        # can run as this window's late filler
                        recip_rows(0, 8)
                    # filler pacing: drain this window's quota evenly
                    for _ in range(POPS[pr]):
                        if filler:
                            filler.popleft()()

            # ---- tail: AV(5) overlapped with qt0/qt1 prefill, then proj ----
            def emit_transposes(qt, prs, psw):
                for p in prs:
                    nc.tensor.matmul(
                        psw[:, p * 128 : (p + 1) * 128],
                        lhsT=waTp[p][:, qt * 128 : (qt + 1) * 128],
                        rhs=ident,
                        start=True,
                        stop=True,
                    )

            def emit_proj(qt, ps, c0, cw, prs, start, stop):
                for i, p in enumerate(prs):
                    nc.tensor.matmul(
                        ps[:, :cw],
                        lhsT=waTp[p][:, qt * 128 : (qt + 1) * 128],
                        rhs=wp[p][:, c0 : c0 + cw],
                        start=start and i == 0,
                        stop=stop and i == len(prs) - 1,
                        skip_group_check=True,
                    )

            def emit_evacs(qt, psw, pss):
                wa_sb = outp.tile([128, D], BF, tag="wa_sb", name="wa_sb")
                nc.scalar.copy(wa_sb, psw[:, :D])
                nc.scalar.dma_start(out=wa_e[qt * 128 : (qt + 1) * 128, :], in_=wa_sb)
                out_sb = outp.tile([128, D], BF, tag="out_sb", name="out_sb")
                nc.vector.tensor_add(out_sb[:, 0:512], pss[0], bpb[:, 0:512])
                nc.vector.tensor_add(
                    out_sb[:, 512:D], pss[1][:, : D - 512], bpb[:, 512:D]
                )
                nc.scalar.dma_start(out=out_e[qt * 128 : (qt + 1) * 128, :], in_=out_sb)

            # prefill qt0/qt1 over pairs 0-4 while AV(5) drains; heads 10-11
            # normalize once AV(5)'s denominators land, then the pair-5
            # columns close each open accumulation
            av5 = deque(av_closures(5))

            def pop_av5(n):
                for _ in range(n):
                    if av5:
                        av5.popleft()()

            # cross-iteration pipelining: the next iteration's input loads
            # and upfront qkT units fill this tail's dependency-chain gaps
            # (the reciprocal chain for heads 10-11 and the qt0/qt1 closes)
            tailf = deque()
            if _it + 1 < unroll:
                nxT, nwqk = alloc_xT_wqk()
                pending["xT"], pending["wqk"] = nxT, nwqk
                tailf.extend(qkT_closures([0, 6], pool=psAV, xT_=nxT, wqk_=nwqk))

            def tail_pop(n):
                for _ in range(n):
                    if tailf:
                        tailf.popleft()()

            pop_av5(8)
            psw0 = psS.tile([128, 1024], F32, tag="ss", name="psw0")
            emit_transposes(0, range(4), psw0)
            pop_av5(8)
            pss0 = [psX.tile([128, 512], F32, tag="px", name="px") for _ in range(2)]
            emit_proj(0, pss0[0], 0, 512, range(4), True, False)
            pop_av5(8)
            emit_proj(0, pss0[1], 512, 256, range(4), True, False)
            pop_av5(4)
            psw1 = psS.tile([128, 1024], F32, tag="ss", name="psw1")
            emit_transposes(1, range(4), psw1)
            emit_transposes(0, [4], psw0)
            emit_proj(0, pss0[0], 0, 512, [4], False, False)
            emit_proj(0, pss0[1], 512, 256, [4], False, False)
            while av5:
                av5.popleft()()
            recip_rows(10, 12)
            tail_pop(12)
            for cl in norm_closures(range(10, H), pool=psAV):
                cl()
            emit_transposes(0, [5], psw0)
            emit_proj(0, pss0[0], 0, 512, [5], False, True)
            emit_proj(0, pss0[1], 512, 256, [5], False, True)
            emit_evacs(0, psw0, pss0)
            tail_pop(4)
            emit_transposes(1, [4, 5], psw1)
            pss1 = [psX.tile([128, 512], F32, tag="px", name="px") for _ in range(2)]
            emit_proj(1, pss1[0], 0, 512, range(H // 2), True, True)
            emit_proj(1, pss1[1], 512, 256, range(H // 2), True, True)
            emit_evacs(1, psw1, pss1)

            for qt in range(2, QT):
                tail_pop(1)
                psw = psS.tile([128, 1024], F32, tag="ss", name="psw")
                emit_transposes(qt, range(H // 2), psw)
                pss = [psX.tile([128, 512], F32, tag="px", name="px") for _ in range(2)]
                emit_proj(qt, pss[0], 0, 512, range(H // 2), True, True)
                emit_proj(qt, pss[1], 512, 256, range(H // 2), True, True)
                emit_evacs(qt, psw, pss)
            while tailf:
                tailf.popleft()()

    if split_waits:
        _prune_implied_waits(nc)
        _split_excess_waits(nc)
    return nc


def make_in_maps(x, w_qkv, b_qkv, w_proj, b_proj):
    """Host-side shard prep: batch element b -> core b; weights replicated."""
    xf = np.asarray(x, dtype=np.float32)
    wqkv = np.asarray(w_qkv, dtype=np.float32)
    bqkv = np.asarray(b_qkv, dtype=np.float32)
    wproj = np.asarray(w_proj, dtype=np.float32)
    bproj = np.asarray(b_proj, dtype=np.float32)

    wqk = np.ascontiguousarray(wqkv[:, : 2 * D]).astype(NP_BF16)
    wv = np.ascontiguousarray(wqkv[:, 2 * D :]).astype(NP_BF16)
    wph = np.ascontiguousarray(wproj.reshape(H // 2, 128, D)).astype(NP_BF16)
    bqk = np.ascontiguousarray(bqkv[: 2 * D].reshape(2 * D // 128, 128).T)
    bv = np.ascontiguousarray(bqkv[2 * D :])
    ident = np.eye(128, dtype=np.float32).astype(NP_BF16)
    selmat = np.kron(np.eye(H, dtype=np.float32), np.ones((1, HD), np.float32)).astype(NP_BF16)

    in_maps = []
    for b in range(N_CORES):
        in_maps.append(
            {
                "xT": np.ascontiguousarray(xf[b].T).astype(NP_BF16),
                "wqk": wqk,
                "wv": wv,
                "wph": wph,
                "bqk": bqk,
                "bv": bv,
                "bp": bproj,
                "ident": ident,
                "selmat": selmat,
            }
        )
    return in_maps


_CACHE = {}


def _get_nc():
    if "nc" not in _CACHE:
        _CACHE["nc"] = build_nc()
    return _CACHE["nc"]


def run_once(in_maps, nc=None):
    """One 8-core execution via the PJRT redirect path (fresh jit per call;
    NEFF comes from the neuron compile cache after the first call)."""
    if nc is None:
        nc = _get_nc()
    return bass2jax.run_bass_via_pjrt(nc, in_maps, n_cores=N_CORES)


def kernel(x, w_qkv, b_qkv, w_proj, b_proj):
    in_maps = make_in_maps(x, w_qkv, b_qkv, w_proj, b_proj)
    results = run_once(in_maps)
    out = np.stack([results[b]["out"] for b in range(N_CORES)]).astype(np.float32)
    wa = np.stack([results[b]["wa"] for b in range(N_CORES)]).astype(np.float32)
    return (out, wa)
